# revision 19
# baseline (speedup 1.0000x reference)
"""Trainium2 Bass kernel for nn_CrossModalAttention (B=2, LQ=LK=2048,
QDIM=HID=1024, KDIM=VDIM=768, H=16, D=64).

Sharding: 8 cores = 2 batches x 4 head-groups (4 heads each).
Per core: q/k/v projections column-sliced over HID, attention for its 4
heads, row-parallel partial of the out-projection. Host sums the 4
partials per batch (the row-parallel unshard) and adds bo.

Device dataflow (per core), all matmuls in fp32r (TF32-like, ~1.5e-4):
  - host passes query/key/value[b] transposed (and K/V key-compacted:
    query_mask masks the KEY axis globally per batch, so masked keys are
    dropped on host and the remainder padded to a multiple of 128)
  - qT/kT [hid, tokens] and v [keys, hid] computed on device
  - per head pair (row-packed K=64 matmuls via tile_position):
    scoresT [keys, q] -> ACT exp(s/8 + mask_bias) -> PV matmul with a
    ones-augmented V (M=65) giving ctxT and the softmax denominator
  - normalize on DVE (reciprocal + gpsimd partition-broadcast)
  - out-projection from ctxT, partial written to DRAM
"""

import math

import ml_dtypes
import numpy as np

B, LQ, LK = 2, 2048, 2048
QDIM, KDIM, VDIM, HID, H = 1024, 768, 768, 1024, 16
D = HID // H  # 64
HG = 4  # head-groups (cores per batch)
HL = H // HG  # heads per core = 4
GH = HL * D  # per-core hid slice = 256
N_CORES = 8
TB = 512  # token block
NTB = LQ // TB  # 4
NEG = -1.0e30

BF16 = True
PROFILE = False
LAST_EXEC_NS = None
LAST_TRACE_DIR = None

_CACHE = {}


def _build(nkt: int, with_bv: bool, bf16: bool):
    import concourse.bacc as bacc
    import concourse.mybir as mybir
    import concourse.tile as tile

    nkeys = nkt * 128
    # key blocks of <=512 for the k-projection
    kbs = [min(512, nkeys - s) for s in range(0, nkeys, 512)]

    f32 = mybir.dt.float32
    f32r = mybir.dt.bfloat16 if bf16 else mybir.dt.float32r
    Exp = mybir.ActivationFunctionType.Exp
    Ident = mybir.ActivationFunctionType.Identity

    nc = bacc.Bacc(
        "TRN2", target_bir_lowering=False, debug=False, num_devices=N_CORES
    )

    # DRAM tensors (per-core shapes)
    XQ = nc.dram_tensor("xq", [128, 8, LQ], f32r, kind="ExternalInput").ap()
    XK = nc.dram_tensor("xk", [128, 6, nkeys], f32r, kind="ExternalInput").ap()
    XV = nc.dram_tensor("xv", [128, 6, nkeys], f32r, kind="ExternalInput").ap()
    WQ = nc.dram_tensor("wq", [128, 8, GH], f32r, kind="ExternalInput").ap()
    WK = nc.dram_tensor("wk", [128, 6, GH], f32r, kind="ExternalInput").ap()
    WV = nc.dram_tensor("wv", [128, 6, GH], f32r, kind="ExternalInput").ap()
    WO = nc.dram_tensor("wo", [128, 2, QDIM], f32r, kind="ExternalInput").ap()
    MB = nc.dram_tensor("mbias", [128, nkt], f32, kind="ExternalInput").ap()
    BQ = nc.dram_tensor("bqk", [128, 4], f32, kind="ExternalInput").ap()
    BV = None
    if with_bv:
        BV = nc.dram_tensor("bv", [128, 2], f32, kind="ExternalInput").ap()
    OUT = nc.dram_tensor("outp", [LQ, QDIM], f32, kind="ExternalOutput").ap()

    with tile.TileContext(nc) as tc:
        with (
            tc.tile_pool(name="consts", bufs=1) as consts,
            tc.tile_pool(name="resid", bufs=1) as resid,
            tc.tile_pool(name="xs", bufs=2) as xs,
            tc.tile_pool(name="probs", bufs=4) as probs_pool,
            tc.tile_pool(name="norm", bufs=3) as norm_pool,
            tc.tile_pool(name="outs", bufs=3) as outs_pool,
            tc.tile_pool(name="ps", bufs=2, space="PSUM") as ps,
        ):
            # ---- constants / weights ----
            # weights go on the gpsimd SWDGE ring so the big input streams
            # (sync HWDGE ring) aren't serialized behind them
            wq_sb = consts.tile([128, 8, GH], f32r)
            wk_sb = consts.tile([128, 6, GH], f32r)
            wv_sb = consts.tile([128, 6, GH], f32r)
            wo_sb = consts.tile([128, 2, QDIM], f32r)
            mb_sb = consts.tile([128, nkt], f32)
            bqk_sb = consts.tile([128, 4], f32)
            nc.gpsimd.dma_start(out=wk_sb, in_=WK)
            nc.gpsimd.dma_start(out=bqk_sb, in_=BQ)
            nc.gpsimd.dma_start(out=mb_sb, in_=MB)
            nc.gpsimd.dma_start(out=wv_sb, in_=WV)
            nc.gpsimd.dma_start(out=wq_sb, in_=WQ)
            nc.gpsimd.dma_start(out=wo_sb, in_=WO)
            bv_sb = None
            if with_bv:
                bv_sb = consts.tile([128, 2], f32)
                nc.gpsimd.dma_start(out=bv_sb, in_=BV)

            # ---- residents ----
            # qT tiles double as ctxT tiles later (WAR handled by Tile)
            qT = [resid.tile([128, LQ], f32r, tag=f"qT{p}", name=f"qT{p}") for p in range(2)]
            kT = [resid.tile([128, nkeys], f32r, tag=f"kT{p}", name=f"kT{p}") for p in range(2)]
            v_sb = resid.tile([128, nkt, HL, D + 1], f32r)
            # ones columns for the denominator rows: fill the whole tile,
            # the v-projection copies then overwrite the [., ., ., 0:D] part
            if bf16:
                nc.vector.memset(v_sb, 1.0)
            else:
                nc.vector.memset(v_sb[:, :, :, :].bitcast(f32), 1.0)

            # ---- k projection: kT[pair][:, key] ----
            for kb_i, kbw in enumerate(kbs):
                s0 = kb_i * 512
                xk_t = xs.tile([128, 6, 512], f32r, tag="xk")
                nc.sync.dma_start(
                    out=xk_t[:, :, :kbw], in_=XK[:, :, s0 : s0 + kbw]
                )
                for m in range(2):
                    ps_t = ps.tile([128, 512], f32, tag="proj")
                    for k in range(6):
                        nc.tensor.matmul(
                            ps_t[:, :kbw],
                            wk_sb[:, k, m * 128 : (m + 1) * 128],
                            xk_t[:, k, :kbw],
                            start=(k == 0),
                            stop=(k == 5),
                        )
                    nc.vector.tensor_scalar_add(
                        kT[m][:, s0 : s0 + kbw],
                        ps_t[:, :kbw],
                        bqk_sb[:, 2 + m : 3 + m],
                    )

            # ---- v projection (emitted later for tb0/p0 split; see
            # emit_vproj) ----
            def emit_vproj():
                for kb_i, kbw in enumerate(kbs):
                    s0 = kb_i * 512
                    xv_t = xs.tile([128, 6, 512], f32r, tag="xv", name="xv_t")
                    nc.sync.dma_start(
                        out=xv_t[:, :, :kbw], in_=XV[:, :, s0 : s0 + kbw]
                    )
                    for sub in range(kbw // 128):
                        kt = (s0 // 128) + sub
                        ps_t = ps.tile(
                            [128, 512], f32, tag="proj", name="vp_ps"
                        )
                        for k in range(6):
                            nc.tensor.matmul(
                                ps_t[:, :GH],
                                xv_t[:, k, sub * 128 : (sub + 1) * 128],
                                wv_sb[:, k, :],
                                start=(k == 0),
                                stop=(k == 5),
                            )
                        nc.vector.tensor_copy(
                            v_sb[:, kt, :, 0:D],
                            ps_t[:, :GH].rearrange("p (h d) -> p h d", h=HL),
                        )

            def emit_qproj(tb):
                t0 = tb * TB
                xq_t = xs.tile([128, 8, TB], f32r, tag="xq", name="xq_t")
                nc.sync.dma_start(out=xq_t, in_=XQ[:, :, t0 : t0 + TB])
                for m in range(2):
                    ps_t = ps.tile([128, 512], f32, tag="proj", name="qp_ps")
                    for k in range(8):
                        nc.tensor.matmul(
                            ps_t,
                            wq_sb[:, k, m * 128 : (m + 1) * 128],
                            xq_t[:, k, :],
                            start=(k == 0),
                            stop=(k == 7),
                        )
                    nc.vector.tensor_scalar_add(
                        qT[m][:, t0 : t0 + TB],
                        ps_t,
                        bqk_sb[:, m : m + 1],
                    )

            def emit_scores(p, tb, kt, prtag, prbufs):
                t0 = tb * TB
                k0 = kt * 128
                sc = ps.tile([128, 2, TB], f32, tag="sc", name="sc")
                for hh in range(2):
                    nc.tensor.matmul(
                        sc[:, hh, :],
                        kT[p][hh * 64 : hh * 64 + 64, k0 : k0 + 128],
                        qT[p][hh * 64 : hh * 64 + 64, t0 : t0 + TB],
                        start=True,
                        stop=True,
                        tile_position=(hh * 64, 0),
                    )
                pr = probs_pool.tile(
                    [128, 2, TB], f32r, tag=prtag, name="pr", bufs=prbufs
                )
                nc.scalar.activation(
                    pr, sc, Exp, bias=mb_sb[:, kt : kt + 1], scale=0.125
                )
                return pr

            def emit_pv(p, tb, kt, pr, ctx_ps):
                for hh in range(2):
                    nc.tensor.matmul(
                        ctx_ps[hh],
                        v_sb[:, kt, 2 * p + hh, :],
                        pr[:, hh, :],
                        start=(kt == 0),
                        stop=(kt == nkt - 1),
                    )

            def emit_normalize(p, tb, ctx_ps):
                t0 = tb * TB
                for hh in range(2):
                    # normalize: denominator row -> partition 0, broadcast,
                    # approx reciprocal on 64 lanes, multiply from PSUM
                    dcp = norm_pool.tile([1, TB], f32, tag="dcp", name="dcp")
                    nc.vector.tensor_copy(dcp, ctx_ps[hh][D : D + 1, :])
                    rbc = norm_pool.tile([D, TB], f32, tag="rbc", name="rbc")
                    nc.gpsimd.partition_broadcast(rbc, dcp)
                    rec = norm_pool.tile([D, TB], f32, tag="rec", name="rec")
                    nc.vector.reciprocal_approx_fast(out=rec, in_=rbc)
                    dst = qT[p][hh * 64 : hh * 64 + 64, t0 : t0 + TB]
                    nc.vector.tensor_mul(dst, ctx_ps[hh][0:D, :], rec)
                    if with_bv:
                        nc.vector.tensor_scalar_add(
                            dst, dst, bv_sb[64 * hh : 64 * hh + 64, p : p + 1]
                        )

            def emit_attn(p, tb):
                ctx_ps = [
                    ps.tile([D + 1, TB], f32, tag="ctx", name=f"ctx{p}_{tb}_{i}")
                    for i in range(2)
                ]
                for kt in range(nkt):
                    pr = emit_scores(p, tb, kt, "pr", 4)
                    emit_pv(p, tb, kt, pr, ctx_ps)
                emit_normalize(p, tb, ctx_ps)

            def emit_outproj(tb):
                for tt in range(4 * tb, 4 * tb + 4):
                    for nh in range(2):
                        ps_t = ps.tile(
                            [128, 512], f32, tag="proj", name="op_ps"
                        )
                        for kk in range(2):
                            nc.tensor.matmul(
                                ps_t,
                                qT[kk][:, tt * 128 : (tt + 1) * 128],
                                wo_sb[:, kk, nh * 512 : (nh + 1) * 512],
                                start=(kk == 0),
                                stop=(kk == 1),
                            )
                        o_sb = outs_pool.tile(
                            [128, 512], f32, tag="osb", name="o_sb"
                        )
                        nc.vector.tensor_copy(o_sb, ps_t)
                        nc.scalar.dma_start(
                            out=OUT[
                                tt * 128 : (tt + 1) * 128,
                                nh * 512 : (nh + 1) * 512,
                            ],
                            in_=o_sb,
                        )

            # ---- emission schedule ----
            # tb0/p0 is split (all scores+exp before the v-projection) so
            # the ACT exp stream starts as early as possible; out-proj of
            # t-block tb is emitted after qproj(tb+1) to hide its
            # normalize dependency
            emit_qproj(0)
            prs = [emit_scores(0, 0, kt, "pr0", nkt) for kt in range(nkt)]
            emit_vproj()
            ctx0 = [
                ps.tile([D + 1, TB], f32, tag="ctx", name=f"ctx00_{i}")
                for i in range(2)
            ]
            for kt in range(nkt):
                emit_pv(0, 0, kt, prs[kt], ctx0)
            emit_normalize(0, 0, ctx0)
            emit_attn(1, 0)
            for tb in range(1, NTB):
                emit_qproj(tb)
                emit_outproj(tb - 1)
                emit_attn(0, tb)
                emit_attn(1, tb)
            emit_outproj(NTB - 1)

    nc.compile()
    return nc


def kernel(
    query, key, value, Wq, bq, Wk, bk, Wv, bv, Wo, bo, query_mask, key_mask
):
    global LAST_EXEC_NS, LAST_TRACE_DIR
    from concourse.bass_utils import run_bass_kernel_spmd

    query = np.asarray(query, dtype=np.float32)
    key = np.asarray(key, dtype=np.float32)
    value = np.asarray(value, dtype=np.float32)
    Wq = np.asarray(Wq, dtype=np.float32)
    Wk = np.asarray(Wk, dtype=np.float32)
    Wv = np.asarray(Wv, dtype=np.float32)
    Wo = np.asarray(Wo, dtype=np.float32)
    bq = np.asarray(bq, dtype=np.float32)
    bk = np.asarray(bk, dtype=np.float32)
    bv = np.asarray(bv, dtype=np.float32)
    bo = np.asarray(bo, dtype=np.float32)
    qm = np.asarray(query_mask)
    km = np.asarray(key_mask)

    # host-side key compaction (query_mask masks the KEY axis, globally
    # per batch)
    keep = [np.flatnonzero(qm[b] != 0) for b in range(B)]
    nkeep = max((len(k) for k in keep), default=0)
    nkt = max(1, math.ceil(nkeep / 128))
    nkeys = nkt * 128

    with_bv = bool(np.any(bv))
    ck = (nkt, with_bv, BF16)
    if ck not in _CACHE:
        _CACHE[ck] = _build(nkt, with_bv, BF16)
    nc = _CACHE[ck]

    wdt = ml_dtypes.bfloat16 if BF16 else np.float32

    def arr_kmajor(a, ktiles):  # [dim, n] -> [128, ktiles, n]
        return np.ascontiguousarray(
            a.reshape(ktiles, 128, a.shape[1]).transpose(1, 0, 2)
        ).astype(wdt)

    in_maps = []
    for c in range(N_CORES):
        b, hg = c // HG, c % HG
        hs = hg * GH
        idx = keep[b]
        # compacted + padded key/value (transposed)
        xk = np.zeros((KDIM, nkeys), np.float32)
        xk[:, : len(idx)] = key[b].T[:, idx]
        xv = np.zeros((VDIM, nkeys), np.float32)
        xv[:, : len(idx)] = value[b].T[:, idx]
        mbias = np.full((nkeys,), NEG, np.float32)
        mbias[: len(idx)] = 0.0
        bqk = np.empty((128, 4), np.float32)
        bqk[:, 0] = bq[hs : hs + 128]
        bqk[:, 1] = bq[hs + 128 : hs + 256]
        bqk[:, 2] = bk[hs : hs + 128]
        bqk[:, 3] = bk[hs + 128 : hs + 256]
        m = {
            "xq": arr_kmajor(query[b].T, 8),
            "xk": arr_kmajor(xk, 6),
            "xv": arr_kmajor(xv, 6),
            "wq": arr_kmajor(Wq[:, hs : hs + GH], 8),
            "wk": arr_kmajor(Wk[:, hs : hs + GH], 6),
            "wv": arr_kmajor(Wv[:, hs : hs + GH], 6),
            "wo": arr_kmajor(Wo[hs : hs + GH, :], 2),
            "mbias": np.ascontiguousarray(mbias.reshape(nkt, 128).T),
            "bqk": bqk,
        }
        if with_bv:
            bvt = np.empty((128, 2), np.float32)
            bvt[:, 0] = bv[hs : hs + 128]
            bvt[:, 1] = bv[hs + 128 : hs + 256]
            m["bv"] = bvt
        in_maps.append(m)

    kwargs = {}
    if PROFILE:
        import tempfile

        LAST_TRACE_DIR = tempfile.mkdtemp(prefix="bass_trace_")
        kwargs = {"trace": True, "tmpdir": LAST_TRACE_DIR}
    res = run_bass_kernel_spmd(nc, in_maps, list(range(N_CORES)), **kwargs)
    LAST_EXEC_NS = res.exec_time_ns

    out = np.zeros((B, LQ, QDIM), np.float32)
    for c in range(N_CORES):
        out[c // HG] += res.results[c]["outp"]
    out += bo[None, None, :]
    for b in range(B):
        if len(keep[b]) == 0:
            # all keys masked: reference softmax is NaN everywhere
            out[b] = np.nan
    # key_mask masks the QUERY axis in the reference; a zero row makes the
    # whole softmax row -inf -> NaN output for that query position.
    for b in range(B):
        zq = np.flatnonzero(km[b] == 0)
        if len(zq):
            out[b, zq, :] = np.nan
    return out


# revision 20
# speedup vs baseline: 1.0912x; 1.0912x over previous
"""Trainium2 Bass kernel for nn_CrossModalAttention (B=2, LQ=LK=2048,
QDIM=HID=1024, KDIM=VDIM=768, H=16, D=64).

Sharding: 8 cores = 2 batches x 4 head-groups (4 heads each).
Per core: q/k/v projections column-sliced over HID, attention for its 4
heads, row-parallel partial of the out-projection. Host sums the 4
partials per batch (the row-parallel unshard) and adds bo.

Device dataflow (per core), all matmuls in fp32r (TF32-like, ~1.5e-4):
  - host passes query/key/value[b] transposed (and K/V key-compacted:
    query_mask masks the KEY axis globally per batch, so masked keys are
    dropped on host and the remainder padded to a multiple of 128)
  - qT/kT [hid, tokens] and v [keys, hid] computed on device
  - per head pair (row-packed K=64 matmuls via tile_position):
    scoresT [keys, q] -> ACT exp(s/8 + mask_bias) -> PV matmul with a
    ones-augmented V (M=65) giving ctxT and the softmax denominator
  - normalize on DVE (reciprocal + gpsimd partition-broadcast)
  - out-projection from ctxT, partial written to DRAM
"""

import math

import ml_dtypes
import numpy as np

B, LQ, LK = 2, 2048, 2048
QDIM, KDIM, VDIM, HID, H = 1024, 768, 768, 1024, 16
D = HID // H  # 64
HG = 4  # head-groups (cores per batch)
HL = H // HG  # heads per core = 4
GH = HL * D  # per-core hid slice = 256
N_CORES = 8
TB = 512  # token block
NTB = LQ // TB  # 4
NEG = -1.0e30

BF16 = True
PROFILE = False
LAST_EXEC_NS = None
LAST_TRACE_DIR = None

_CACHE = {}


def _build(nkt: int, with_bv: bool, bf16: bool):
    import concourse.bacc as bacc
    import concourse.mybir as mybir
    import concourse.tile as tile

    nkeys = nkt * 128
    # key blocks of <=512 for the k-projection
    kbs = [min(512, nkeys - s) for s in range(0, nkeys, 512)]

    f32 = mybir.dt.float32
    f32r = mybir.dt.bfloat16 if bf16 else mybir.dt.float32r
    Exp = mybir.ActivationFunctionType.Exp
    Ident = mybir.ActivationFunctionType.Identity

    nc = bacc.Bacc(
        "TRN2", target_bir_lowering=False, debug=False, num_devices=N_CORES
    )

    # DRAM tensors (per-core shapes)
    XQ = nc.dram_tensor("xq", [128, 8, LQ], f32r, kind="ExternalInput").ap()
    XK = nc.dram_tensor("xk", [128, 6, nkeys], f32r, kind="ExternalInput").ap()
    XV = nc.dram_tensor("xv", [128, 6, nkeys], f32r, kind="ExternalInput").ap()
    WQ = nc.dram_tensor("wq", [128, 8, GH], f32r, kind="ExternalInput").ap()
    WK = nc.dram_tensor("wk", [128, 6, GH], f32r, kind="ExternalInput").ap()
    WV = nc.dram_tensor("wv", [128, 6, GH], f32r, kind="ExternalInput").ap()
    WO = nc.dram_tensor("wo", [128, 2, QDIM], f32r, kind="ExternalInput").ap()
    MB = nc.dram_tensor("mbias", [128, nkt], f32, kind="ExternalInput").ap()
    BQ = nc.dram_tensor("bqk", [128, 4], f32, kind="ExternalInput").ap()
    BV = None
    if with_bv:
        BV = nc.dram_tensor("bv", [128, 2], f32, kind="ExternalInput").ap()
    OUT = nc.dram_tensor("outp", [LQ, QDIM], f32, kind="ExternalOutput").ap()

    with tile.TileContext(nc) as tc:
        with (
            tc.tile_pool(name="consts", bufs=1) as consts,
            tc.tile_pool(name="resid", bufs=1) as resid,
            tc.tile_pool(name="xs", bufs=2) as xs,
            tc.tile_pool(name="probs", bufs=4) as probs_pool,
            tc.tile_pool(name="norm", bufs=3) as norm_pool,
            tc.tile_pool(name="outs", bufs=3) as outs_pool,
            tc.tile_pool(name="ps", bufs=2, space="PSUM") as ps,
        ):
            # ---- constants / weights ----
            # weights go on the gpsimd SWDGE ring so the big input streams
            # (sync HWDGE ring) aren't serialized behind them
            wq_sb = consts.tile([128, 8, GH], f32r)
            wk_sb = consts.tile([128, 6, GH], f32r)
            wv_sb = consts.tile([128, 6, GH], f32r)
            wo_sb = consts.tile([128, 2, QDIM], f32r)
            mb_sb = consts.tile([128, nkt], f32)
            bqk_sb = consts.tile([128, 4], f32)
            nc.gpsimd.dma_start(out=wk_sb, in_=WK)
            nc.gpsimd.dma_start(out=bqk_sb, in_=BQ)
            nc.gpsimd.dma_start(out=mb_sb, in_=MB)
            nc.gpsimd.dma_start(out=wv_sb, in_=WV)
            nc.gpsimd.dma_start(out=wq_sb, in_=WQ)
            nc.gpsimd.dma_start(out=wo_sb, in_=WO)
            bv_sb = None
            if with_bv:
                bv_sb = consts.tile([128, 2], f32)
                nc.gpsimd.dma_start(out=bv_sb, in_=BV)

            # ---- residents ----
            # qT tiles double as ctxT tiles later (WAR handled by Tile)
            qT = [resid.tile([128, LQ], f32r, tag=f"qT{p}", name=f"qT{p}") for p in range(2)]
            kT = [resid.tile([128, nkeys], f32r, tag=f"kT{p}", name=f"kT{p}") for p in range(2)]
            v_sb = resid.tile([128, nkt, HL, D + 1], f32r)
            # ones columns for the denominator rows: fill the whole tile,
            # the v-projection copies then overwrite the [., ., ., 0:D] part
            if bf16:
                nc.vector.memset(v_sb, 1.0)
            else:
                nc.vector.memset(v_sb[:, :, :, :].bitcast(f32), 1.0)

            # ---- k projection (per key-block, so attention can begin
            # after kb0) ----
            def emit_kproj(kb_i):
                kbw = kbs[kb_i]
                s0 = kb_i * 512
                xk_t = xs.tile([128, 6, 512], f32r, tag="xk", name="xk_t")
                nc.sync.dma_start(
                    out=xk_t[:, :, :kbw], in_=XK[:, :, s0 : s0 + kbw]
                )
                for m in range(2):
                    ps_t = ps.tile([128, 512], f32, tag="proj", name="kp_ps")
                    for k in range(6):
                        nc.tensor.matmul(
                            ps_t[:, :kbw],
                            wk_sb[:, k, m * 128 : (m + 1) * 128],
                            xk_t[:, k, :kbw],
                            start=(k == 0),
                            stop=(k == 5),
                        )
                    nc.vector.tensor_scalar_add(
                        kT[m][:, s0 : s0 + kbw],
                        ps_t[:, :kbw],
                        bqk_sb[:, 2 + m : 3 + m],
                    )

            # ---- v projection (emitted later for tb0/p0 split; see
            # emit_vproj) ----
            def emit_vproj():
                for kb_i, kbw in enumerate(kbs):
                    s0 = kb_i * 512
                    xv_t = xs.tile([128, 6, 512], f32r, tag="xv", name="xv_t")
                    nc.sync.dma_start(
                        out=xv_t[:, :, :kbw], in_=XV[:, :, s0 : s0 + kbw]
                    )
                    for sub in range(kbw // 128):
                        kt = (s0 // 128) + sub
                        ps_t = ps.tile(
                            [128, 512], f32, tag="proj", name="vp_ps"
                        )
                        for k in range(6):
                            nc.tensor.matmul(
                                ps_t[:, :GH],
                                xv_t[:, k, sub * 128 : (sub + 1) * 128],
                                wv_sb[:, k, :],
                                start=(k == 0),
                                stop=(k == 5),
                            )
                        nc.vector.tensor_copy(
                            v_sb[:, kt, :, 0:D],
                            ps_t[:, :GH].rearrange("p (h d) -> p h d", h=HL),
                        )

            def emit_qproj(tb):
                t0 = tb * TB
                xq_t = xs.tile([128, 8, TB], f32r, tag="xq", name="xq_t")
                nc.sync.dma_start(out=xq_t, in_=XQ[:, :, t0 : t0 + TB])
                for m in range(2):
                    ps_t = ps.tile([128, 512], f32, tag="proj", name="qp_ps")
                    for k in range(8):
                        nc.tensor.matmul(
                            ps_t,
                            wq_sb[:, k, m * 128 : (m + 1) * 128],
                            xq_t[:, k, :],
                            start=(k == 0),
                            stop=(k == 7),
                        )
                    nc.vector.tensor_scalar_add(
                        qT[m][:, t0 : t0 + TB],
                        ps_t,
                        bqk_sb[:, m : m + 1],
                    )

            def emit_scores(p, tb, kt, prtag, prbufs):
                t0 = tb * TB
                k0 = kt * 128
                sc = ps.tile([128, 2, TB], f32, tag="sc", name="sc")
                for hh in range(2):
                    nc.tensor.matmul(
                        sc[:, hh, :],
                        kT[p][hh * 64 : hh * 64 + 64, k0 : k0 + 128],
                        qT[p][hh * 64 : hh * 64 + 64, t0 : t0 + TB],
                        start=True,
                        stop=True,
                        tile_position=(hh * 64, 0),
                    )
                pr = probs_pool.tile(
                    [128, 2, TB], f32r, tag=prtag, name="pr", bufs=prbufs
                )
                nc.scalar.activation(
                    pr, sc, Exp, bias=mb_sb[:, kt : kt + 1], scale=0.125
                )
                return pr

            def emit_pv(p, tb, kt, pr, ctx_ps):
                for hh in range(2):
                    nc.tensor.matmul(
                        ctx_ps[hh],
                        v_sb[:, kt, 2 * p + hh, :],
                        pr[:, hh, :],
                        start=(kt == 0),
                        stop=(kt == nkt - 1),
                    )

            def emit_normalize(p, tb, ctx_ps):
                t0 = tb * TB
                for hh in range(2):
                    # normalize: denominator row -> partition 0, broadcast,
                    # approx reciprocal on 64 lanes, multiply from PSUM
                    dcp = norm_pool.tile([1, TB], f32, tag="dcp", name="dcp")
                    nc.vector.tensor_copy(dcp, ctx_ps[hh][D : D + 1, :])
                    rbc = norm_pool.tile([D, TB], f32, tag="rbc", name="rbc")
                    nc.gpsimd.partition_broadcast(rbc, dcp)
                    rec = norm_pool.tile([D, TB], f32, tag="rec", name="rec")
                    nc.vector.reciprocal_approx_fast(out=rec, in_=rbc)
                    dst = qT[p][hh * 64 : hh * 64 + 64, t0 : t0 + TB]
                    nc.vector.tensor_mul(dst, ctx_ps[hh][0:D, :], rec)
                    if with_bv:
                        nc.vector.tensor_scalar_add(
                            dst, dst, bv_sb[64 * hh : 64 * hh + 64, p : p + 1]
                        )

            def emit_attn(p, tb):
                ctx_ps = [
                    ps.tile([D + 1, TB], f32, tag="ctx", name=f"ctx{p}_{tb}_{i}")
                    for i in range(2)
                ]
                for kt in range(nkt):
                    pr = emit_scores(p, tb, kt, "pr", 4)
                    emit_pv(p, tb, kt, pr, ctx_ps)
                emit_normalize(p, tb, ctx_ps)

            def emit_outproj(tb):
                for tt in range(4 * tb, 4 * tb + 4):
                    for nh in range(2):
                        ps_t = ps.tile(
                            [128, 512], f32, tag="proj", name="op_ps"
                        )
                        for kk in range(2):
                            nc.tensor.matmul(
                                ps_t,
                                qT[kk][:, tt * 128 : (tt + 1) * 128],
                                wo_sb[:, kk, nh * 512 : (nh + 1) * 512],
                                start=(kk == 0),
                                stop=(kk == 1),
                            )
                        o_sb = outs_pool.tile(
                            [128, 512], f32, tag="osb", name="o_sb"
                        )
                        nc.vector.tensor_copy(o_sb, ps_t)
                        nc.sync.dma_start(
                            out=OUT[
                                tt * 128 : (tt + 1) * 128,
                                nh * 512 : (nh + 1) * 512,
                            ],
                            in_=o_sb,
                        )

            # ---- emission schedule ----
            # tb0/p0 is split (all scores+exp before the v-projection) so
            # the ACT exp stream starts as early as possible; out-proj of
            # t-block tb is emitted after qproj(tb+1) to hide its
            # normalize dependency
            emit_kproj(0)
            emit_qproj(0)
            kt_kb0 = min(4, nkt)
            prs = [emit_scores(0, 0, kt, "pr0", nkt) for kt in range(kt_kb0)]
            for kb_i in range(1, len(kbs)):
                emit_kproj(kb_i)
            prs += [emit_scores(0, 0, kt, "pr0", nkt) for kt in range(kt_kb0, nkt)]
            emit_vproj()
            ctx0 = [
                ps.tile([D + 1, TB], f32, tag="ctx", name=f"ctx00_{i}")
                for i in range(2)
            ]
            for kt in range(nkt):
                emit_pv(0, 0, kt, prs[kt], ctx0)
            emit_normalize(0, 0, ctx0)
            emit_attn(1, 0)
            for tb in range(1, NTB):
                emit_qproj(tb)
                emit_outproj(tb - 1)
                emit_attn(0, tb)
                emit_attn(1, tb)
            emit_outproj(NTB - 1)

    nc.compile()
    return nc


def kernel(
    query, key, value, Wq, bq, Wk, bk, Wv, bv, Wo, bo, query_mask, key_mask
):
    global LAST_EXEC_NS, LAST_TRACE_DIR
    from concourse.bass_utils import run_bass_kernel_spmd

    query = np.asarray(query, dtype=np.float32)
    key = np.asarray(key, dtype=np.float32)
    value = np.asarray(value, dtype=np.float32)
    Wq = np.asarray(Wq, dtype=np.float32)
    Wk = np.asarray(Wk, dtype=np.float32)
    Wv = np.asarray(Wv, dtype=np.float32)
    Wo = np.asarray(Wo, dtype=np.float32)
    bq = np.asarray(bq, dtype=np.float32)
    bk = np.asarray(bk, dtype=np.float32)
    bv = np.asarray(bv, dtype=np.float32)
    bo = np.asarray(bo, dtype=np.float32)
    qm = np.asarray(query_mask)
    km = np.asarray(key_mask)

    # host-side key compaction (query_mask masks the KEY axis, globally
    # per batch)
    keep = [np.flatnonzero(qm[b] != 0) for b in range(B)]
    nkeep = max((len(k) for k in keep), default=0)
    nkt = max(1, math.ceil(nkeep / 128))
    nkeys = nkt * 128

    with_bv = bool(np.any(bv))
    ck = (nkt, with_bv, BF16)
    if ck not in _CACHE:
        _CACHE[ck] = _build(nkt, with_bv, BF16)
    nc = _CACHE[ck]

    wdt = ml_dtypes.bfloat16 if BF16 else np.float32

    def arr_kmajor(a, ktiles):  # [dim, n] -> [128, ktiles, n]
        return np.ascontiguousarray(
            a.reshape(ktiles, 128, a.shape[1]).transpose(1, 0, 2)
        ).astype(wdt)

    in_maps = []
    for c in range(N_CORES):
        b, hg = c // HG, c % HG
        hs = hg * GH
        idx = keep[b]
        # compacted + padded key/value (transposed)
        xk = np.zeros((KDIM, nkeys), np.float32)
        xk[:, : len(idx)] = key[b].T[:, idx]
        xv = np.zeros((VDIM, nkeys), np.float32)
        xv[:, : len(idx)] = value[b].T[:, idx]
        mbias = np.full((nkeys,), NEG, np.float32)
        mbias[: len(idx)] = 0.0
        bqk = np.empty((128, 4), np.float32)
        bqk[:, 0] = bq[hs : hs + 128]
        bqk[:, 1] = bq[hs + 128 : hs + 256]
        bqk[:, 2] = bk[hs : hs + 128]
        bqk[:, 3] = bk[hs + 128 : hs + 256]
        m = {
            "xq": arr_kmajor(query[b].T, 8),
            "xk": arr_kmajor(xk, 6),
            "xv": arr_kmajor(xv, 6),
            "wq": arr_kmajor(Wq[:, hs : hs + GH], 8),
            "wk": arr_kmajor(Wk[:, hs : hs + GH], 6),
            "wv": arr_kmajor(Wv[:, hs : hs + GH], 6),
            "wo": arr_kmajor(Wo[hs : hs + GH, :], 2),
            "mbias": np.ascontiguousarray(mbias.reshape(nkt, 128).T),
            "bqk": bqk,
        }
        if with_bv:
            bvt = np.empty((128, 2), np.float32)
            bvt[:, 0] = bv[hs : hs + 128]
            bvt[:, 1] = bv[hs + 128 : hs + 256]
            m["bv"] = bvt
        in_maps.append(m)

    kwargs = {}
    if PROFILE:
        import tempfile

        LAST_TRACE_DIR = tempfile.mkdtemp(prefix="bass_trace_")
        kwargs = {"trace": True, "tmpdir": LAST_TRACE_DIR}
    res = run_bass_kernel_spmd(nc, in_maps, list(range(N_CORES)), **kwargs)
    LAST_EXEC_NS = res.exec_time_ns

    out = np.zeros((B, LQ, QDIM), np.float32)
    for c in range(N_CORES):
        out[c // HG] += res.results[c]["outp"]
    out += bo[None, None, :]
    for b in range(B):
        if len(keep[b]) == 0:
            # all keys masked: reference softmax is NaN everywhere
            out[b] = np.nan
    # key_mask masks the QUERY axis in the reference; a zero row makes the
    # whole softmax row -inf -> NaN output for that query position.
    for b in range(B):
        zq = np.flatnonzero(km[b] == 0)
        if len(zq):
            out[b, zq, :] = np.nan
    return out
